# revision 24
# baseline (speedup 1.0000x reference)
"""DCNv2 (modulated deformable conv) Trainium2 kernel, v3.

8 cores = 4 batch samples x 2 image halves. Per core:
  1. Offset conv on PE (bf16 moving): om[27, 8192] -> omdram (bf16).
  2. Hat planes: Aneg[d] = -relu(1-|dy-(d-2)|)*sigmoid(m) on [36, 2176]
     tiles; t-plane = (dx + 3 + kx + j) in fp16, (k, w8, j) layout,
     written to DRAM row-major for per-row broadcasts.
  3. Per 4-row block: texp broadcast (115 parts), amt broadcast (Aneg,
     23x replication per d), then batched M-build:
     ACT Abs(texp,bias=-x_l) -> DVE (min(sab,1)-1) -> DVE mult amt.
  4. Stage-1: 24 matmuls (115-contraction) per row into ONE h2-packed
     psum supertile [128, 1024] (tile_position col-packing); single
     612-col strided evac into vt [128, 612] (psum col order),
     rotating Pool/ACT/DVE.
  5. Stage-2: per row 18 matmuls w2r2[(c,h2),128].T @ vt 4-dim slices
     into row-triple psum [128, 3*136]; ACT adds bias on evac.
"""
import sys
sys.path.insert(0, "/opt/trn_rl_repo")
import numpy as np
import concourse.bass as bass
import concourse.tile as tile
from concourse import bacc, mybir
from concourse.bass_utils import run_bass_kernel_spmd

F32, BF16, FP16 = mybir.dt.float32, mybir.dt.bfloat16, mybir.dt.float16
ALU = mybir.AluOpType
AF = mybir.ActivationFunctionType

B, C, O, H, W = 4, 64, 128, 128, 128
K, HH = 9, 64
P = HH * W
CW, NW, ND = 17, 8, 5
XW = 23                # x-window width (dx in [-1.6, 1.6] + kx + bilinear)
NP = ND * XW           # 115 contraction partitions
WJ = NW * CW           # 136
N1 = K * WJ            # 1224
RG = HH + 2            # 66 xwin rowgroups
XSL = 16               # xwb ring slots
NCORES = 8

_cache = {}


def _ap(base, dims):
    """Manual AP: keep base partition dim, replace free dims."""
    return bass.AP(base.tensor, base.offset, [base.ap[0]] + dims)


def build_bass(debug=False):
    nc = bacc.Bacc("TRN2", target_bir_lowering=False, debug=False,
                   num_devices=NCORES)
    dp = lambda n, s, dt, out=False: nc.dram_tensor(
        n, s, dt, kind="ExternalOutput" if out else "ExternalInput").ap()

    feat_d = dp("feat", [2 * C, RG * 130], BF16)
    xwin_d = dp("xwin", [NP, RG * NW * C], BF16)
    womr_d = dp("womr", [2 * C, 9 * 27], BF16)
    w2r_d = dp("w2r", [2 * C, K * O], BF16)
    jkt_d = dp("jkt", [72, 1088], FP16)
    bias_d = dp("bias", [O, 1], F32)
    bom_d = dp("bom", [27, 1], F32)
    dbias_d = dp("dbias", [72, 5], F32)
    mxlb_d = dp("mxlb", [NP, 1], F32)
    out_d = dp("out", [O, P], BF16, out=True)

    omdram = nc.dram_tensor("omdram", [27, P], BF16).ap()
    tdram = nc.dram_tensor("tdram", [HH * N1], FP16).ap()
    adram = nc.dram_tensor("adram", [HH * ND * N1], BF16).ap()
    t_wv = tdram.rearrange("(r k c) -> k r c", k=K, c=WJ)       # write view
    a_wv = adram.rearrange("(r d k c) -> d k r c", d=ND, k=K, c=WJ)

    with tile.TileContext(nc) as tc:
        with (
            tc.tile_pool(name="small", bufs=1) as small,
            tc.tile_pool(name="big", bufs=1) as big,
            tc.tile_pool(name="plp", bufs=2) as plp,
            tc.tile_pool(name="io", bufs=4) as io,
            tc.tile_pool(name="rowp", bufs=3) as rowp,
            tc.tile_pool(name="mrp", bufs=2) as mrp,
            tc.tile_pool(name="vpool", bufs=6) as vpool,
            tc.tile_pool(name="ps", bufs=2, space="PSUM") as ps,
            tc.tile_pool(name="ps2", bufs=1, space="PSUM") as ps2,
        ):
            womr = small.tile([2 * C, 9 * 27], BF16)
            nc.sync.dma_start(out=womr[:], in_=womr_d[:])
            w2r = small.tile([2 * C, K * O], BF16)
            nc.sync.dma_start(out=w2r[:], in_=w2r_d[:])
            jkt2, dbias2 = [], []
            for hf in range(2):
                jh = small.tile([36, 1088], FP16, name=f"jkt{hf}")
                nc.sync.dma_start(out=jh[:],
                                  in_=jkt_d[hf * 36:(hf + 1) * 36, :])
                jkt2.append(jh)
            bias = small.tile([O, 1], F32)
            nc.sync.dma_start(out=bias[:], in_=bias_d[:])
            bom = small.tile([27, 1], F32)
            nc.sync.dma_start(out=bom[:], in_=bom_d[:])
            for hf in range(2):
                dh = small.tile([36, 5], F32, name=f"dbias{hf}")
                nc.sync.dma_start(out=dh[:],
                                  in_=dbias_d[hf * 36:(hf + 1) * 36, :])
                dbias2.append(dh)
            mxlb = small.tile([NP, 1], F32)
            nc.sync.dma_start(out=mxlb[:], in_=mxlb_d[:])

            feat = big.tile([2 * C, RG * 130], BF16)
            nc.sync.dma_start(out=feat[:], in_=feat_d[:])
            xwb = big.tile([NP, XSL * NW * C], BF16)
            nc.sync.dma_start(out=xwb[:, 0:10 * NW * C],
                              in_=xwin_d[:, 0:10 * NW * C])
            loaded = 10

            # ---- 1+2. offset conv + hat planes, per image half so rows
            #           can start while the second half is still cooking ----
            def om_conv(chunks):
                for ch in chunks:
                    pom = ps.tile([2 * C, 1024], F32, tag="sup")
                    for t9 in range(9):
                        dy, dx = t9 // 3, t9 % 3
                        base = feat[:, (ch * 4 + dy) * 130 + dx:
                                    (ch * 4 + dy) * 130 + dx + 1]
                        rhs = _ap(base, [[130, 4], [1, 128]])
                        nc.tensor.matmul(pom[0:27, 0:512],
                                         womr[:, t9 * 27:(t9 + 1) * 27],
                                         rhs, start=(t9 == 0), stop=(t9 == 8))
                    omc = io.tile([27, 512], BF16, tag="omc")
                    nc.scalar.activation(omc[:], pom[0:27, 0:512],
                                         AF.Identity, bias=bom[:])
                    nc.sync.dma_start(out=omdram[:, ch * 512:(ch + 1) * 512],
                                      in_=omc[:])

            def planes(hf):
                # hf in {0,1}: rows hf*32 .. hf*32+31 (quarters 2hf, 2hf+1)
                dyf = plp.tile([36, 1032], BF16, tag="dyf")
                dxf = plp.tile([36, 1032], BF16, tag="dxf")
                msf = plp.tile([36, 1032], BF16, tag="msf")
                for f in (dyf, dxf, msf):
                    nc.vector.memset(f[:, 1024:1032], 0.0)
                for qh in range(4):                      # (q-in-half, hq)
                    pix = slice(hf * 4096 + qh * 1024,
                                hf * 4096 + qh * 1024 + 1024)
                    pr = qh * 9
                    nc.sync.dma_start(out=dyf[pr:pr + 9, 0:1024],
                                      in_=omdram[0:18:2, pix])
                    nc.sync.dma_start(out=dxf[pr:pr + 9, 0:1024],
                                      in_=omdram[1:18:2, pix])
                    nc.sync.dma_start(out=msf[pr:pr + 9, 0:1024],
                                      in_=omdram[18:27, pix])
                jk = jkt2[hf][:]
                db = dbias2[hf]
                nc.scalar.activation(msf[:, 0:1024], msf[:, 0:1024],
                                     AF.Sigmoid, bias=db[:, 2:3])
                dxv = _ap(dxf[:], [[128, 8], [17, 8], [1, 17]])
                dyv = _ap(dyf[:], [[128, 8], [17, 8], [1, 17]])
                msv = _ap(msf[:], [[128, 8], [17, 8], [1, 17]])
                t136 = plp.tile([36, 1088], FP16, tag="t136")
                nc.vector.tensor_tensor(
                    t136[:].rearrange("p (a b c) -> p a b c", a=8, b=8, c=17),
                    dxv, jk.rearrange("p (a b c) -> p a b c",
                                      a=8, b=8, c=17), op=ALU.add)
                for qh in range(4):
                    r0 = hf * 32 + qh * 8
                    nc.sync.dma_start(
                        out=t_wv[:, r0:r0 + 8, :],
                        in_=t136[qh * 9:qh * 9 + 9, :].rearrange(
                            "p (r c) -> p r c", r=8))
                for d5 in range(ND):
                    ab = plp.tile([36, 1088], BF16, tag="aplane")
                    ab4 = ab[:].rearrange("p (a b c) -> p a b c",
                                          a=8, b=8, c=17)
                    nc.scalar.activation(ab4, dyv, AF.Abs,
                                         bias=db[:, d5:d5 + 1], scale=1.0)
                    nc.vector.tensor_scalar(ab[:], ab[:], 1.0, None,
                                            op0=ALU.subtract)
                    abb = plp.tile([36, 1088], BF16, tag="aplaneb")
                    abb4 = abb[:].rearrange("p (a b c) -> p a b c",
                                            a=8, b=8, c=17)
                    nc.vector.tensor_tensor(abb4, ab4, msv, op=ALU.mult)
                    # NEGATED A-plane: Aneg = min(abb, 0) = -relu(-abb)
                    nc.vector.tensor_scalar(abb[:], abb[:], 0.0, None,
                                            op0=ALU.min)
                    for qh in range(4):
                        r0 = hf * 32 + qh * 8
                        nc.sync.dma_start(
                            out=a_wv[d5, :, r0:r0 + 8, :],
                            in_=abb[qh * 9:qh * 9 + 9, :].rearrange(
                                "p (r c) -> p r c", r=8))

            om_conv(range(0, 8))
            planes(0)
            om_conv(range(8, 16))

            # ---- 3-5. rows ----
            pend = []
            tri = {}
            vts = {}
            mrows = {}

            def stage2(r):
                # tri-psum [O, 1024]: h2=0 groups in bank0 (cols 68*idx),
                # h2=1 in bank1 (cols 512+68*idx) -- mixed contraction
                # row-groups must not share a psum bank.
                bl = r % 8
                idx = bl % 3 if bl < 6 else bl - 6
                if idx == 0:
                    tri["p"] = ps2.tile([O, 1024], F32, tag="pout",
                                        name="pout")
                    tri["r0"] = r
                pout = tri["p"]
                vt = vts.pop(r)
                for h2 in range(2):
                    col0 = 512 * h2 + 68 * idx
                    for k9 in range(K):
                        rbase = vt[h2 * C:(h2 + 1) * C,
                                   17 * k9:17 * k9 + 1]
                        mv = _ap(rbase, [[306, 2], [153, 2], [1, 17]])
                        nc.tensor.matmul(
                            pout[:, col0:col0 + 68],
                            w2r[h2 * C:(h2 + 1) * C, k9 * O:(k9 + 1) * O],
                            mv, start=(k9 == 0), stop=(k9 == K - 1))
                if bl in (2, 5, 7):
                    n = idx + 1
                    osb = io.tile([O, 3 * WJ], BF16, tag="osb")
                    sbase = pout[:, 0:1]
                    src = _ap(sbase, [[512, 2], [68, n], [1, 68]])
                    nc.scalar.activation(osb[:, 0:n * WJ], src, AF.Identity,
                                         bias=bias[:])
                    r0 = tri["r0"]
                    nc.sync.dma_start(
                        out=bass.AP(out_d.tensor, out_d.offset + r0 * 128,
                                    [out_d.ap[0], [128, n], [1, 68]]),
                        in_=_ap(osb[:, 0:1], [[68, n], [1, 68]]))
                    nc.sync.dma_start(
                        out=bass.AP(out_d.tensor,
                                    out_d.offset + r0 * 128 + 68,
                                    [out_d.ap[0], [128, n], [1, 60]]),
                        in_=_ap(osb[:, n * 68:n * 68 + 1],
                                [[68, n], [1, 60]]))

            for r in range(HH):
                if r == 4:
                    planes(1)
                if r % 8 == 0 and r > 0:
                    need = min(r + 9, RG - 1)
                    while loaded <= need:
                        s0 = loaded % XSL
                        cnt = min(XSL - s0, need - loaded + 1)
                        nc.sync.dma_start(
                            out=xwb[:, s0 * 512:(s0 + cnt) * 512],
                            in_=xwin_d[:, loaded * 512:(loaded + cnt) * 512])
                        loaded += cnt
                if r % 4 == 0:
                    texp4 = rowp.tile([NP, 4 * N1], FP16, tag="texp")
                    nc.gpsimd.dma_start(
                        out=texp4[:],
                        in_=tdram[r * N1:(r + 4) * N1]
                        .unsqueeze(0).broadcast_to([NP, 4 * N1]))
                    amt4 = rowp.tile([NP, 4 * N1], BF16, tag="amt")
                    for d5 in range(ND):
                        nc.gpsimd.dma_start(
                            out=amt4[d5 * XW:(d5 + 1) * XW, :].rearrange(
                                "p (w c) -> p w c", w=4),
                            in_=bass.AP(adram.tensor,
                                        adram.offset + (r * ND + d5) * N1,
                                        [[0, XW], [ND * N1, 4], [1, N1]]))
                    # batched M-build for 4 rows:
                    # M = relu(1-|t-x|)*A = (min(|t-x|,1)-1) * (-A)
                    sab4 = mrp.tile([NP, 4 * N1], BF16, tag="sab")
                    nc.scalar.activation(sab4[:], texp4[:], AF.Abs,
                                         bias=mxlb[:], scale=1.0)
                    nc.vector.tensor_scalar(sab4[:], sab4[:], 1.0, 1.0,
                                            op0=ALU.min, op1=ALU.subtract)
                    mrow4 = mrp.tile([NP, 4 * N1], BF16, tag="mrow")
                    nc.vector.tensor_tensor(mrow4[:], sab4[:], amt4[:],
                                            op=ALU.mult)
                    mrows[r // 4] = mrow4
                mrow4 = mrows[r // 4]
                moff = (r % 4) * N1
                vt = vpool.tile([2 * C, 612], BF16, tag="v")
                vts[r] = vt
                sup = ps.tile([2 * C, 1024], F32, tag="sup")
                for h2 in range(2):
                    for g in range(4):
                        w8 = h2 * 4 + g
                        off = (g // 2) * 512 + (g % 2) * 153
                        for ky in range(3):
                            slot = (r + ky) % XSL
                            st = xwb[:, slot * 512 + w8 * C:
                                     slot * 512 + (w8 + 1) * C]
                            mbase = mrow4[:, moff + (3 * ky) * WJ + w8 * CW:
                                          moff + (3 * ky) * WJ + w8 * CW + 1]
                            mv = _ap(mbase, [[WJ, 3], [1, CW]])
                            nc.tensor.matmul(
                                sup[h2 * C:(h2 + 1) * C,
                                    off + 51 * ky:off + 51 * (ky + 1)],
                                st, mv, start=True, stop=True,
                                tile_position=(0, h2 * C))
                # single strided evac [128, 612]: psum (b,s,run) -> vt
                sbase = sup[:, 0:1]
                src = _ap(sbase, [[512, 2], [153, 2], [1, 153]])
                if r % 2 == 0:
                    nc.vector.tensor_copy(vt[:], src)
                else:
                    nc.scalar.activation(vt[:], src, AF.Copy, bias=0.0)
                pend.append(r)
                if r >= 2:
                    stage2(pend.pop(0))
            while pend:
                stage2(pend.pop(0))
    nc.compile()
    return nc


def host_prep(input_feat, inter, weight, bias, w_om, b_om):
    import ml_dtypes
    bf = ml_dtypes.bfloat16
    maps = []
    womr = np.ascontiguousarray(
        w_om.transpose(1, 2, 3, 0).reshape(2 * C, 9 * 27)).astype(bf)
    w2r1 = np.ascontiguousarray(
        weight.reshape(O, C, K).transpose(1, 2, 0).reshape(C, K * O))
    w2r = np.concatenate([w2r1, w2r1], axis=0).astype(bf)   # h2-duplicated
    kk = (np.arange(K) % 3).astype(np.float32)
    jj = np.arange(CW, dtype=np.float32)
    # jkt tile layout: [72=(q,h,k), (r8, w8, j)] -> value dep on (k, j)
    jkt_t = np.zeros((72, 1088), np.float32)
    kwj = (3.0 + kk[:, None, None] + 0 * np.arange(NW)[None, :, None]
           + jj[None, None, :])                          # [9, 8, 17]
    for qh in range(8):
        for k in range(K):
            jkt_t[qh * 9 + k] = np.tile(kwj[k].reshape(1, NW * CW),
                                        (8, 1)).reshape(-1)
    dbias = np.tile(-(np.arange(5, dtype=np.float32) - 2), (72, 1))
    # x-window slot l ( = p % XW ) sits at t-coordinate l+1
    mxlb = -(np.arange(NP) % XW + 1).astype(np.float32)[:, None]
    for b in range(B):
        xpad = np.zeros((C, H + 6, 144), np.float32)
        xpad[:, 3:3 + H, 4:4 + W] = input_feat[b]
        featb = np.concatenate([input_feat[b], inter[b]], axis=0)
        for half in range(2):
            h0 = half * HH
            fs = np.zeros((2 * C, RG, 130), np.float32)
            r_lo, r_hi = max(0, h0 - 1), min(H, h0 + 65)
            fs[:, r_lo - (h0 - 1):r_hi - (h0 - 1), 1:129] = featb[:, r_lo:r_hi]
            xs = xpad[:, h0:h0 + 70, :]                    # [C, 70, 144]
            rows = (np.arange(RG)[:, None, None, None]
                    + np.arange(ND)[None, None, None, :])  # r + d
            cols = (17 * np.arange(NW)[None, :, None, None] + 1
                    + np.arange(XW)[None, None, :, None])
            xw = xs[:, rows, cols]                         # [C,66,NW,XW,ND]
            xw = xw.transpose(4, 3, 1, 2, 0).reshape(NP, RG * NW * C)
            maps.append({
                "feat": fs.reshape(2 * C, RG * 130).astype(bf),
                "xwin": xw.astype(bf),
                "womr": womr, "w2r": w2r,
                "jkt": jkt_t.astype(np.float16),
                "bias": np.asarray(bias, np.float32).reshape(O, 1),
                "bom": np.asarray(b_om, np.float32).reshape(27, 1),
                "dbias": dbias,
                "mxlb": mxlb,
            })
    return maps


def kernel(input_feat, inter, weight, bias, w_om, b_om):
    if "nc" not in _cache:
        _cache["nc"] = build_bass()
    nc = _cache["nc"]
    maps = host_prep(np.asarray(input_feat, np.float32),
                     np.asarray(inter, np.float32),
                     np.asarray(weight, np.float32),
                     np.asarray(bias, np.float32),
                     np.asarray(w_om, np.float32),
                     np.asarray(b_om, np.float32))
    res = run_bass_kernel_spmd(nc, maps, list(range(NCORES)))
    out = np.zeros((B, O, H, W), np.float32)
    for ci in range(NCORES):
        b, half = ci // 2, ci % 2
        out[b, :, half * HH:(half + 1) * HH] = \
            res.results[ci]["out"].astype(np.float32).reshape(O, HH, W)
    return out


# revision 25
# speedup vs baseline: 1.0464x; 1.0464x over previous
"""DCNv2 (modulated deformable conv) Trainium2 kernel, v3.

8 cores = 4 batch samples x 2 image halves. Per core:
  1. Offset conv on PE (bf16 moving): om[27, 8192] -> omdram (bf16).
  2. Hat planes: Aneg[d] = -relu(1-|dy-(d-2)|)*sigmoid(m) on [36, 2176]
     tiles; t-plane = (dx + 3 + kx + j) in fp16, (k, w8, j) layout,
     written to DRAM row-major for per-row broadcasts.
  3. Per 4-row block: texp broadcast (115 parts), amt broadcast (Aneg,
     23x replication per d), then batched M-build:
     ACT Abs(texp,bias=-x_l) -> DVE (min(sab,1)-1) -> DVE mult amt.
  4. Stage-1: 24 matmuls (115-contraction) per row into ONE h2-packed
     psum supertile [128, 1024] (tile_position col-packing); single
     612-col strided evac into vt [128, 612] (psum col order),
     rotating Pool/ACT/DVE.
  5. Stage-2: per row 18 matmuls w2r2[(c,h2),128].T @ vt 4-dim slices
     into row-triple psum [128, 3*136]; ACT adds bias on evac.
"""
import sys
sys.path.insert(0, "/opt/trn_rl_repo")
import numpy as np
import concourse.bass as bass
import concourse.tile as tile
from concourse import bacc, mybir
from concourse.bass_utils import run_bass_kernel_spmd

F32, BF16, FP16 = mybir.dt.float32, mybir.dt.bfloat16, mybir.dt.float16
ALU = mybir.AluOpType
AF = mybir.ActivationFunctionType

B, C, O, H, W = 4, 64, 128, 128, 128
K, HH = 9, 64
P = HH * W
CW, NW, ND = 17, 8, 5
XW = 23                # x-window width (dx in [-1.6, 1.6] + kx + bilinear)
NP = ND * XW           # 115 contraction partitions
WJ = NW * CW           # 136
N1 = K * WJ            # 1224
RG = HH + 2            # 66 xwin rowgroups
XSL = 16               # xwb ring slots
NCORES = 8

_cache = {}


def _ap(base, dims):
    """Manual AP: keep base partition dim, replace free dims."""
    return bass.AP(base.tensor, base.offset, [base.ap[0]] + dims)


def build_bass(debug=False):
    nc = bacc.Bacc("TRN2", target_bir_lowering=False, debug=False,
                   num_devices=NCORES)
    dp = lambda n, s, dt, out=False: nc.dram_tensor(
        n, s, dt, kind="ExternalOutput" if out else "ExternalInput").ap()

    feat_d = dp("feat", [2 * C, RG * 130], BF16)
    xwin_d = dp("xwin", [NP, RG * NW * C], BF16)
    womr_d = dp("womr", [2 * C, 9 * 27], BF16)
    w2r_d = dp("w2r", [2 * C, K * O], BF16)
    jkt_d = dp("jkt", [72, 1088], FP16)
    bias_d = dp("bias", [O, 1], F32)
    bom_d = dp("bom", [27, 1], F32)
    dbias_d = dp("dbias", [72, 5], F32)
    mxlb_d = dp("mxlb", [NP, 1], F32)
    out_d = dp("out", [O, P], BF16, out=True)

    omdram = nc.dram_tensor("omdram", [27, P], BF16).ap()
    tdram = nc.dram_tensor("tdram", [HH * N1], FP16).ap()
    adram = nc.dram_tensor("adram", [HH * ND * N1], BF16).ap()
    t_wv = tdram.rearrange("(r k c) -> k r c", k=K, c=WJ)       # write view
    a_wv = adram.rearrange("(r d k c) -> d k r c", d=ND, k=K, c=WJ)

    with tile.TileContext(nc) as tc:
        with (
            tc.tile_pool(name="small", bufs=1) as small,
            tc.tile_pool(name="big", bufs=1) as big,
            tc.tile_pool(name="plp", bufs=2) as plp,
            tc.tile_pool(name="io", bufs=4) as io,
            tc.tile_pool(name="rowp", bufs=3) as rowp,
            tc.tile_pool(name="mrp", bufs=2) as mrp,
            tc.tile_pool(name="vpool", bufs=6) as vpool,
            tc.tile_pool(name="ps", bufs=3, space="PSUM") as ps,
            tc.tile_pool(name="ps2", bufs=1, space="PSUM") as ps2,
        ):
            womr = small.tile([2 * C, 9 * 27], BF16)
            nc.sync.dma_start(out=womr[:], in_=womr_d[:])
            w2r = small.tile([2 * C, K * O], BF16)
            nc.sync.dma_start(out=w2r[:], in_=w2r_d[:])
            jkt2, dbias2 = [], []
            for hf in range(2):
                jh = small.tile([36, 1088], FP16, name=f"jkt{hf}")
                nc.sync.dma_start(out=jh[:],
                                  in_=jkt_d[hf * 36:(hf + 1) * 36, :])
                jkt2.append(jh)
            bias = small.tile([O, 1], F32)
            nc.sync.dma_start(out=bias[:], in_=bias_d[:])
            bom = small.tile([27, 1], F32)
            nc.sync.dma_start(out=bom[:], in_=bom_d[:])
            for hf in range(2):
                dh = small.tile([36, 5], F32, name=f"dbias{hf}")
                nc.sync.dma_start(out=dh[:],
                                  in_=dbias_d[hf * 36:(hf + 1) * 36, :])
                dbias2.append(dh)
            mxlb = small.tile([NP, 1], F32)
            nc.sync.dma_start(out=mxlb[:], in_=mxlb_d[:])

            feat = big.tile([2 * C, RG * 130], BF16)
            nc.sync.dma_start(out=feat[:], in_=feat_d[:])
            xwb = big.tile([NP, XSL * NW * C], BF16)
            nc.sync.dma_start(out=xwb[:, 0:10 * NW * C],
                              in_=xwin_d[:, 0:10 * NW * C])
            loaded = 10

            # ---- 1+2. offset conv + hat planes, per image half so rows
            #           can start while the second half is still cooking ----
            def om_conv(chunks):
                for ch in chunks:
                    pom = ps.tile([2 * C, 1024], F32, tag="sup")
                    for t9 in range(9):
                        dy, dx = t9 // 3, t9 % 3
                        base = feat[:, (ch * 4 + dy) * 130 + dx:
                                    (ch * 4 + dy) * 130 + dx + 1]
                        rhs = _ap(base, [[130, 4], [1, 128]])
                        nc.tensor.matmul(pom[0:27, 0:512],
                                         womr[:, t9 * 27:(t9 + 1) * 27],
                                         rhs, start=(t9 == 0), stop=(t9 == 8))
                    omc = io.tile([27, 512], BF16, tag="omc")
                    nc.scalar.activation(omc[:], pom[0:27, 0:512],
                                         AF.Identity, bias=bom[:])
                    nc.sync.dma_start(out=omdram[:, ch * 512:(ch + 1) * 512],
                                      in_=omc[:])

            def planes(hf):
                # hf in {0,1}: rows hf*32 .. hf*32+31 (quarters 2hf, 2hf+1)
                dyf = plp.tile([36, 1032], BF16, tag="dyf")
                dxf = plp.tile([36, 1032], BF16, tag="dxf")
                msf = plp.tile([36, 1032], BF16, tag="msf")
                for f in (dyf, dxf, msf):
                    nc.vector.memset(f[:, 1024:1032], 0.0)
                for qh in range(4):                      # (q-in-half, hq)
                    pix = slice(hf * 4096 + qh * 1024,
                                hf * 4096 + qh * 1024 + 1024)
                    pr = qh * 9
                    nc.sync.dma_start(out=dyf[pr:pr + 9, 0:1024],
                                      in_=omdram[0:18:2, pix])
                    nc.sync.dma_start(out=dxf[pr:pr + 9, 0:1024],
                                      in_=omdram[1:18:2, pix])
                    nc.sync.dma_start(out=msf[pr:pr + 9, 0:1024],
                                      in_=omdram[18:27, pix])
                jk = jkt2[hf][:]
                db = dbias2[hf]
                nc.scalar.activation(msf[:, 0:1024], msf[:, 0:1024],
                                     AF.Sigmoid, bias=db[:, 2:3])
                dxv = _ap(dxf[:], [[128, 8], [17, 8], [1, 17]])
                dyv = _ap(dyf[:], [[128, 8], [17, 8], [1, 17]])
                msv = _ap(msf[:], [[128, 8], [17, 8], [1, 17]])
                t136 = plp.tile([36, 1088], FP16, tag="t136")
                nc.vector.tensor_tensor(
                    t136[:].rearrange("p (a b c) -> p a b c", a=8, b=8, c=17),
                    dxv, jk.rearrange("p (a b c) -> p a b c",
                                      a=8, b=8, c=17), op=ALU.add)
                for qh in range(4):
                    r0 = hf * 32 + qh * 8
                    nc.sync.dma_start(
                        out=t_wv[:, r0:r0 + 8, :],
                        in_=t136[qh * 9:qh * 9 + 9, :].rearrange(
                            "p (r c) -> p r c", r=8))
                for d5 in range(ND):
                    ab = plp.tile([36, 1088], BF16, tag="aplane")
                    ab4 = ab[:].rearrange("p (a b c) -> p a b c",
                                          a=8, b=8, c=17)
                    nc.scalar.activation(ab4, dyv, AF.Abs,
                                         bias=db[:, d5:d5 + 1], scale=1.0)
                    nc.vector.tensor_scalar(ab[:], ab[:], 1.0, None,
                                            op0=ALU.subtract)
                    abb = plp.tile([36, 1088], BF16, tag="aplaneb")
                    abb4 = abb[:].rearrange("p (a b c) -> p a b c",
                                            a=8, b=8, c=17)
                    nc.vector.tensor_tensor(abb4, ab4, msv, op=ALU.mult)
                    # NEGATED A-plane: Aneg = min(abb, 0) = -relu(-abb)
                    nc.vector.tensor_scalar(abb[:], abb[:], 0.0, None,
                                            op0=ALU.min)
                    for qh in range(4):
                        r0 = hf * 32 + qh * 8
                        nc.sync.dma_start(
                            out=a_wv[d5, :, r0:r0 + 8, :],
                            in_=abb[qh * 9:qh * 9 + 9, :].rearrange(
                                "p (r c) -> p r c", r=8))

            om_conv(range(0, 8))
            planes(0)
            om_conv(range(8, 16))

            # ---- 3-5. rows ----
            pend = []
            tri = {}
            vts = {}
            mrows = {}

            def stage2(r):
                # tri-psum [O, 1024]: h2=0 groups in bank0 (cols 68*idx),
                # h2=1 in bank1 (cols 512+68*idx) -- mixed contraction
                # row-groups must not share a psum bank.
                bl = r % 8
                idx = bl % 3 if bl < 6 else bl - 6
                if idx == 0:
                    tri["p"] = ps2.tile([O, 1024], F32, tag="pout",
                                        name="pout")
                    tri["r0"] = r
                pout = tri["p"]
                vt = vts.pop(r)
                for h2 in range(2):
                    col0 = 512 * h2 + 68 * idx
                    for k9 in range(K):
                        rbase = vt[h2 * C:(h2 + 1) * C,
                                   17 * k9:17 * k9 + 1]
                        mv = _ap(rbase, [[306, 2], [153, 2], [1, 17]])
                        nc.tensor.matmul(
                            pout[:, col0:col0 + 68],
                            w2r[h2 * C:(h2 + 1) * C, k9 * O:(k9 + 1) * O],
                            mv, start=(k9 == 0), stop=(k9 == K - 1))
                if bl in (2, 5, 7):
                    n = idx + 1
                    osb = io.tile([O, 3 * WJ], BF16, tag="osb")
                    sbase = pout[:, 0:1]
                    src = _ap(sbase, [[512, 2], [68, n], [1, 68]])
                    nc.scalar.activation(osb[:, 0:n * WJ], src, AF.Identity,
                                         bias=bias[:])
                    r0 = tri["r0"]
                    nc.sync.dma_start(
                        out=bass.AP(out_d.tensor, out_d.offset + r0 * 128,
                                    [out_d.ap[0], [128, n], [1, 68]]),
                        in_=_ap(osb[:, 0:1], [[68, n], [1, 68]]))
                    nc.sync.dma_start(
                        out=bass.AP(out_d.tensor,
                                    out_d.offset + r0 * 128 + 68,
                                    [out_d.ap[0], [128, n], [1, 60]]),
                        in_=_ap(osb[:, n * 68:n * 68 + 1],
                                [[68, n], [1, 60]]))

            for r in range(HH):
                if r == 4:
                    planes(1)
                if r % 8 == 0 and r > 0:
                    need = min(r + 9, RG - 1)
                    while loaded <= need:
                        s0 = loaded % XSL
                        cnt = min(XSL - s0, need - loaded + 1)
                        nc.sync.dma_start(
                            out=xwb[:, s0 * 512:(s0 + cnt) * 512],
                            in_=xwin_d[:, loaded * 512:(loaded + cnt) * 512])
                        loaded += cnt
                if r % 4 == 0:
                    texp4 = rowp.tile([NP, 4 * N1], FP16, tag="texp")
                    nc.gpsimd.dma_start(
                        out=texp4[:],
                        in_=tdram[r * N1:(r + 4) * N1]
                        .unsqueeze(0).broadcast_to([NP, 4 * N1]))
                    amt4 = rowp.tile([NP, 4 * N1], BF16, tag="amt")
                    for d5 in range(ND):
                        nc.gpsimd.dma_start(
                            out=amt4[d5 * XW:(d5 + 1) * XW, :].rearrange(
                                "p (w c) -> p w c", w=4),
                            in_=bass.AP(adram.tensor,
                                        adram.offset + (r * ND + d5) * N1,
                                        [[0, XW], [ND * N1, 4], [1, N1]]))
                    # batched M-build for 4 rows:
                    # M = relu(1-|t-x|)*A = (min(|t-x|,1)-1) * (-A)
                    sab4 = mrp.tile([NP, 4 * N1], BF16, tag="sab")
                    nc.scalar.activation(sab4[:], texp4[:], AF.Abs,
                                         bias=mxlb[:], scale=1.0)
                    nc.vector.tensor_scalar(sab4[:], sab4[:], 1.0, 1.0,
                                            op0=ALU.min, op1=ALU.subtract)
                    mrow4 = mrp.tile([NP, 4 * N1], BF16, tag="mrow")
                    nc.vector.tensor_tensor(mrow4[:], sab4[:], amt4[:],
                                            op=ALU.mult)
                    mrows[r // 4] = mrow4
                mrow4 = mrows[r // 4]
                moff = (r % 4) * N1
                vt = vpool.tile([2 * C, 612], BF16, tag="v")
                vts[r] = vt
                sup = ps.tile([2 * C, 1024], F32, tag="sup")
                for h2 in range(2):
                    for g in range(4):
                        w8 = h2 * 4 + g
                        off = (g // 2) * 512 + (g % 2) * 153
                        for ky in range(3):
                            slot = (r + ky) % XSL
                            st = xwb[:, slot * 512 + w8 * C:
                                     slot * 512 + (w8 + 1) * C]
                            mbase = mrow4[:, moff + (3 * ky) * WJ + w8 * CW:
                                          moff + (3 * ky) * WJ + w8 * CW + 1]
                            mv = _ap(mbase, [[WJ, 3], [1, CW]])
                            nc.tensor.matmul(
                                sup[h2 * C:(h2 + 1) * C,
                                    off + 51 * ky:off + 51 * (ky + 1)],
                                st, mv, start=True, stop=True,
                                tile_position=(0, h2 * C))
                # single strided evac [128, 612]: psum (b,s,run) -> vt
                sbase = sup[:, 0:1]
                src = _ap(sbase, [[512, 2], [153, 2], [1, 153]])
                if r % 2 == 0:
                    nc.vector.tensor_copy(vt[:], src)
                else:
                    nc.scalar.activation(vt[:], src, AF.Copy, bias=0.0)
                pend.append(r)
                if r >= 1:
                    stage2(pend.pop(0))
            while pend:
                stage2(pend.pop(0))
    nc.compile()
    return nc


def host_prep(input_feat, inter, weight, bias, w_om, b_om):
    import ml_dtypes
    bf = ml_dtypes.bfloat16
    maps = []
    womr = np.ascontiguousarray(
        w_om.transpose(1, 2, 3, 0).reshape(2 * C, 9 * 27)).astype(bf)
    w2r1 = np.ascontiguousarray(
        weight.reshape(O, C, K).transpose(1, 2, 0).reshape(C, K * O))
    w2r = np.concatenate([w2r1, w2r1], axis=0).astype(bf)   # h2-duplicated
    kk = (np.arange(K) % 3).astype(np.float32)
    jj = np.arange(CW, dtype=np.float32)
    # jkt tile layout: [72=(q,h,k), (r8, w8, j)] -> value dep on (k, j)
    jkt_t = np.zeros((72, 1088), np.float32)
    kwj = (3.0 + kk[:, None, None] + 0 * np.arange(NW)[None, :, None]
           + jj[None, None, :])                          # [9, 8, 17]
    for qh in range(8):
        for k in range(K):
            jkt_t[qh * 9 + k] = np.tile(kwj[k].reshape(1, NW * CW),
                                        (8, 1)).reshape(-1)
    dbias = np.tile(-(np.arange(5, dtype=np.float32) - 2), (72, 1))
    # x-window slot l ( = p % XW ) sits at t-coordinate l+1
    mxlb = -(np.arange(NP) % XW + 1).astype(np.float32)[:, None]
    for b in range(B):
        xpad = np.zeros((C, H + 6, 144), np.float32)
        xpad[:, 3:3 + H, 4:4 + W] = input_feat[b]
        featb = np.concatenate([input_feat[b], inter[b]], axis=0)
        for half in range(2):
            h0 = half * HH
            fs = np.zeros((2 * C, RG, 130), np.float32)
            r_lo, r_hi = max(0, h0 - 1), min(H, h0 + 65)
            fs[:, r_lo - (h0 - 1):r_hi - (h0 - 1), 1:129] = featb[:, r_lo:r_hi]
            xs = xpad[:, h0:h0 + 70, :]                    # [C, 70, 144]
            rows = (np.arange(RG)[:, None, None, None]
                    + np.arange(ND)[None, None, None, :])  # r + d
            cols = (17 * np.arange(NW)[None, :, None, None] + 1
                    + np.arange(XW)[None, None, :, None])
            xw = xs[:, rows, cols]                         # [C,66,NW,XW,ND]
            xw = xw.transpose(4, 3, 1, 2, 0).reshape(NP, RG * NW * C)
            maps.append({
                "feat": fs.reshape(2 * C, RG * 130).astype(bf),
                "xwin": xw.astype(bf),
                "womr": womr, "w2r": w2r,
                "jkt": jkt_t.astype(np.float16),
                "bias": np.asarray(bias, np.float32).reshape(O, 1),
                "bom": np.asarray(b_om, np.float32).reshape(27, 1),
                "dbias": dbias,
                "mxlb": mxlb,
            })
    return maps


def kernel(input_feat, inter, weight, bias, w_om, b_om):
    if "nc" not in _cache:
        _cache["nc"] = build_bass()
    nc = _cache["nc"]
    maps = host_prep(np.asarray(input_feat, np.float32),
                     np.asarray(inter, np.float32),
                     np.asarray(weight, np.float32),
                     np.asarray(bias, np.float32),
                     np.asarray(w_om, np.float32),
                     np.asarray(b_om, np.float32))
    res = run_bass_kernel_spmd(nc, maps, list(range(NCORES)))
    out = np.zeros((B, O, H, W), np.float32)
    for ci in range(NCORES):
        b, half = ci // 2, ci % 2
        out[b, :, half * HH:(half + 1) * HH] = \
            res.results[ci]["out"].astype(np.float32).reshape(O, HH, W)
    return out


# revision 26
# speedup vs baseline: 1.0601x; 1.0131x over previous
"""DCNv2 (modulated deformable conv) Trainium2 kernel, v3.

8 cores = 4 batch samples x 2 image halves. Per core:
  1. Offset conv on PE (bf16 moving): om[27, 8192] -> omdram (bf16).
  2. Hat planes: Aneg[d] = -relu(1-|dy-(d-2)|)*sigmoid(m) on [36, 2176]
     tiles; t-plane = (dx + 3 + kx + j) in fp16, (k, w8, j) layout,
     written to DRAM row-major for per-row broadcasts.
  3. Per 4-row block: texp broadcast (115 parts), amt broadcast (Aneg,
     23x replication per d), then batched M-build:
     ACT Abs(texp,bias=-x_l) -> DVE (min(sab,1)-1) -> DVE mult amt.
  4. Stage-1: 24 matmuls (115-contraction) per row into ONE h2-packed
     psum supertile [128, 1024] (tile_position col-packing); single
     612-col strided evac into vt [128, 612] (psum col order),
     rotating Pool/ACT/DVE.
  5. Stage-2: per row 18 matmuls w2r2[(c,h2),128].T @ vt 4-dim slices
     into row-triple psum [128, 3*136]; ACT adds bias on evac.
"""
import sys
sys.path.insert(0, "/opt/trn_rl_repo")
import numpy as np
import concourse.bass as bass
import concourse.tile as tile
from concourse import bacc, mybir
from concourse.bass_utils import run_bass_kernel_spmd

F32, BF16, FP16 = mybir.dt.float32, mybir.dt.bfloat16, mybir.dt.float16
ALU = mybir.AluOpType
AF = mybir.ActivationFunctionType

B, C, O, H, W = 4, 64, 128, 128, 128
K, HH = 9, 64
P = HH * W
CW, NW, ND = 17, 8, 5
XW = 23                # x-window width (dx in [-1.6, 1.6] + kx + bilinear)
NP = ND * XW           # 115 contraction partitions
WJ = NW * CW           # 136
N1 = K * WJ            # 1224
RG = HH + 2            # 66 xwin rowgroups
XSL = 16               # xwb ring slots
NCORES = 8

_cache = {}


def _ap(base, dims):
    """Manual AP: keep base partition dim, replace free dims."""
    return bass.AP(base.tensor, base.offset, [base.ap[0]] + dims)


def build_bass(debug=False):
    nc = bacc.Bacc("TRN2", target_bir_lowering=False, debug=False,
                   num_devices=NCORES)
    dp = lambda n, s, dt, out=False: nc.dram_tensor(
        n, s, dt, kind="ExternalOutput" if out else "ExternalInput").ap()

    feat_d = dp("feat", [2 * C, RG * 130], BF16)
    xwin_d = dp("xwin", [NP, RG * NW * C], BF16)
    womr_d = dp("womr", [2 * C, 9 * 27], BF16)
    w2r_d = dp("w2r", [2 * C, K * O], BF16)
    jkt_d = dp("jkt", [72, 1088], FP16)
    bias_d = dp("bias", [O, 1], F32)
    bom_d = dp("bom", [27, 1], F32)
    dbias_d = dp("dbias", [72, 5], F32)
    mxlb_d = dp("mxlb", [NP, 1], F32)
    out_d = dp("out", [O, P], BF16, out=True)

    omdram = nc.dram_tensor("omdram", [27, P], BF16).ap()
    tdram = nc.dram_tensor("tdram", [HH * N1], FP16).ap()
    adram = nc.dram_tensor("adram", [HH * ND * N1], BF16).ap()
    t_wv = tdram.rearrange("(r k c) -> k r c", k=K, c=WJ)       # write view
    a_wv = adram.rearrange("(r d k c) -> d k r c", d=ND, k=K, c=WJ)

    with tile.TileContext(nc) as tc:
        with (
            tc.tile_pool(name="small", bufs=1) as small,
            tc.tile_pool(name="big", bufs=1) as big,
            tc.tile_pool(name="plp", bufs=2) as plp,
            tc.tile_pool(name="io", bufs=4) as io,
            tc.tile_pool(name="rowp", bufs=3) as rowp,
            tc.tile_pool(name="mrp", bufs=2) as mrp,
            tc.tile_pool(name="vpool", bufs=6) as vpool,
            tc.tile_pool(name="ps", bufs=3, space="PSUM") as ps,
            tc.tile_pool(name="ps2", bufs=1, space="PSUM") as ps2,
        ):
            womr = small.tile([2 * C, 9 * 27], BF16)
            nc.sync.dma_start(out=womr[:], in_=womr_d[:])
            w2r = small.tile([2 * C, K * O], BF16)
            nc.sync.dma_start(out=w2r[:], in_=w2r_d[:])
            jkt2, dbias2 = [], []
            for hf in range(2):
                jh = small.tile([36, 1088], FP16, name=f"jkt{hf}")
                nc.sync.dma_start(out=jh[:],
                                  in_=jkt_d[hf * 36:(hf + 1) * 36, :])
                jkt2.append(jh)
            bias = small.tile([O, 1], F32)
            nc.sync.dma_start(out=bias[:], in_=bias_d[:])
            bom = small.tile([27, 1], F32)
            nc.sync.dma_start(out=bom[:], in_=bom_d[:])
            for hf in range(2):
                dh = small.tile([36, 5], F32, name=f"dbias{hf}")
                nc.sync.dma_start(out=dh[:],
                                  in_=dbias_d[hf * 36:(hf + 1) * 36, :])
                dbias2.append(dh)
            mxlb = small.tile([NP, 1], F32)
            nc.sync.dma_start(out=mxlb[:], in_=mxlb_d[:])

            feat = big.tile([2 * C, RG * 130], BF16)
            nc.sync.dma_start(out=feat[:], in_=feat_d[:])
            xwb = big.tile([NP, XSL * NW * C], BF16)
            nc.sync.dma_start(out=xwb[:, 0:10 * NW * C],
                              in_=xwin_d[:, 0:10 * NW * C])
            loaded = 10

            # ---- 1+2. offset conv + hat planes, per image half so rows
            #           can start while the second half is still cooking ----
            def om_conv(chunks):
                for ch in chunks:
                    pom = ps.tile([2 * C, 1024], F32, tag="sup")
                    for t9 in range(9):
                        dy, dx = t9 // 3, t9 % 3
                        base = feat[:, (ch * 4 + dy) * 130 + dx:
                                    (ch * 4 + dy) * 130 + dx + 1]
                        rhs = _ap(base, [[130, 4], [1, 128]])
                        nc.tensor.matmul(pom[0:27, 0:512],
                                         womr[:, t9 * 27:(t9 + 1) * 27],
                                         rhs, start=(t9 == 0), stop=(t9 == 8))
                    omc = io.tile([27, 512], BF16, tag="omc")
                    nc.scalar.activation(omc[:], pom[0:27, 0:512],
                                         AF.Identity, bias=bom[:])
                    nc.sync.dma_start(out=omdram[:, ch * 512:(ch + 1) * 512],
                                      in_=omc[:])

            def planes(hf):
                # hf in {0,1}: rows hf*32 .. hf*32+31 (quarters 2hf, 2hf+1)
                dyf = plp.tile([36, 1032], BF16, tag="dyf")
                dxf = plp.tile([36, 1032], BF16, tag="dxf")
                msf = plp.tile([36, 1032], BF16, tag="msf")
                for f in (dyf, dxf, msf):
                    nc.vector.memset(f[:, 1024:1032], 0.0)
                for qh in range(4):                      # (q-in-half, hq)
                    pix = slice(hf * 4096 + qh * 1024,
                                hf * 4096 + qh * 1024 + 1024)
                    pr = qh * 9
                    nc.sync.dma_start(out=dyf[pr:pr + 9, 0:1024],
                                      in_=omdram[0:18:2, pix])
                    nc.sync.dma_start(out=dxf[pr:pr + 9, 0:1024],
                                      in_=omdram[1:18:2, pix])
                    nc.sync.dma_start(out=msf[pr:pr + 9, 0:1024],
                                      in_=omdram[18:27, pix])
                jk = jkt2[hf][:]
                db = dbias2[hf]
                nc.scalar.activation(msf[:, 0:1024], msf[:, 0:1024],
                                     AF.Sigmoid, bias=db[:, 2:3])
                dxv = _ap(dxf[:], [[128, 8], [17, 8], [1, 17]])
                dyv = _ap(dyf[:], [[128, 8], [17, 8], [1, 17]])
                msv = _ap(msf[:], [[128, 8], [17, 8], [1, 17]])
                t136 = plp.tile([36, 1088], FP16, tag="t136")
                nc.vector.tensor_tensor(
                    t136[:].rearrange("p (a b c) -> p a b c", a=8, b=8, c=17),
                    dxv, jk.rearrange("p (a b c) -> p a b c",
                                      a=8, b=8, c=17), op=ALU.add)
                for qh in range(4):
                    r0 = hf * 32 + qh * 8
                    nc.sync.dma_start(
                        out=t_wv[:, r0:r0 + 8, :],
                        in_=t136[qh * 9:qh * 9 + 9, :].rearrange(
                            "p (r c) -> p r c", r=8))
                for d5 in range(ND):
                    ab = plp.tile([36, 1088], BF16, tag="aplane")
                    ab4 = ab[:].rearrange("p (a b c) -> p a b c",
                                          a=8, b=8, c=17)
                    nc.scalar.activation(ab4, dyv, AF.Abs,
                                         bias=db[:, d5:d5 + 1], scale=1.0)
                    nc.vector.tensor_scalar(ab[:], ab[:], 1.0, None,
                                            op0=ALU.subtract)
                    abb = plp.tile([36, 1088], BF16, tag="aplaneb")
                    abb4 = abb[:].rearrange("p (a b c) -> p a b c",
                                            a=8, b=8, c=17)
                    nc.vector.tensor_tensor(abb4, ab4, msv, op=ALU.mult)
                    # NEGATED A-plane: Aneg = min(abb, 0) = -relu(-abb)
                    nc.vector.tensor_scalar(abb[:], abb[:], 0.0, None,
                                            op0=ALU.min)
                    for qh in range(4):
                        r0 = hf * 32 + qh * 8
                        nc.sync.dma_start(
                            out=a_wv[d5, :, r0:r0 + 8, :],
                            in_=abb[qh * 9:qh * 9 + 9, :].rearrange(
                                "p (r c) -> p r c", r=8))

            om_conv(range(0, 8))
            planes(0)
            om_conv(range(8, 16))

            # ---- 3-5. rows ----
            pend = []
            tri = {}
            vts = {}
            mrows = {}

            def stage2(r):
                # tri-psum [O, 1024]: h2=0 groups in bank0 (cols 68*idx),
                # h2=1 in bank1 (cols 512+68*idx) -- mixed contraction
                # row-groups must not share a psum bank.
                bl = r % 8
                idx = bl % 3 if bl < 6 else bl - 6
                if idx == 0:
                    tri["p"] = ps2.tile([O, 1024], F32, tag="pout",
                                        name="pout")
                    tri["r0"] = r
                pout = tri["p"]
                vt = vts.pop(r)
                for h2 in range(2):
                    col0 = 512 * h2 + 68 * idx
                    for k9 in range(K):
                        rbase = vt[h2 * C:(h2 + 1) * C,
                                   17 * k9:17 * k9 + 1]
                        mv = _ap(rbase, [[306, 2], [153, 2], [1, 17]])
                        nc.tensor.matmul(
                            pout[:, col0:col0 + 68],
                            w2r[h2 * C:(h2 + 1) * C, k9 * O:(k9 + 1) * O],
                            mv, start=(k9 == 0), stop=(k9 == K - 1))
                if bl in (2, 5, 7):
                    n = idx + 1
                    osb = io.tile([O, 3 * WJ], BF16, tag="osb")
                    sbase = pout[:, 0:1]
                    src = _ap(sbase, [[512, 2], [68, n], [1, 68]])
                    nc.scalar.activation(osb[:, 0:n * WJ], src, AF.Identity,
                                         bias=bias[:])
                    r0 = tri["r0"]
                    nc.sync.dma_start(
                        out=bass.AP(out_d.tensor, out_d.offset + r0 * 128,
                                    [out_d.ap[0], [128, n], [1, 68]]),
                        in_=_ap(osb[:, 0:1], [[68, n], [1, 68]]))
                    nc.sync.dma_start(
                        out=bass.AP(out_d.tensor,
                                    out_d.offset + r0 * 128 + 68,
                                    [out_d.ap[0], [128, n], [1, 60]]),
                        in_=_ap(osb[:, n * 68:n * 68 + 1],
                                [[68, n], [1, 60]]))

            for r in range(HH):
                if r == 4:
                    planes(1)
                if r % 8 == 0 and r > 0:
                    need = min(r + 9, RG - 1)
                    while loaded <= need:
                        s0 = loaded % XSL
                        cnt = min(XSL - s0, need - loaded + 1)
                        nc.sync.dma_start(
                            out=xwb[:, s0 * 512:(s0 + cnt) * 512],
                            in_=xwin_d[:, loaded * 512:(loaded + cnt) * 512])
                        loaded += cnt
                if r % 4 == 0:
                    texp4 = rowp.tile([NP, 4 * N1], FP16, tag="texp")
                    nc.gpsimd.dma_start(
                        out=texp4[:],
                        in_=tdram[r * N1:(r + 4) * N1]
                        .unsqueeze(0).broadcast_to([NP, 4 * N1]))
                    amt4 = rowp.tile([NP, 4 * N1], BF16, tag="amt")
                    for d5 in range(ND):
                        nc.gpsimd.dma_start(
                            out=amt4[d5 * XW:(d5 + 1) * XW, :].rearrange(
                                "p (w c) -> p w c", w=4),
                            in_=bass.AP(adram.tensor,
                                        adram.offset + (r * ND + d5) * N1,
                                        [[0, XW], [ND * N1, 4], [1, N1]]))
                    # batched M-build for 4 rows:
                    # M = relu(1-|t-x|)*A = (min(|t-x|,1)-1) * (-A)
                    sab4 = mrp.tile([NP, 4 * N1], BF16, tag="sab")
                    nc.scalar.activation(sab4[:], texp4[:], AF.Abs,
                                         bias=mxlb[:], scale=1.0)
                    nc.vector.tensor_scalar(sab4[:], sab4[:], 1.0, 1.0,
                                            op0=ALU.min, op1=ALU.subtract)
                    mrow4 = mrp.tile([NP, 4 * N1], BF16, tag="mrow")
                    nc.vector.tensor_tensor(mrow4[:], sab4[:], amt4[:],
                                            op=ALU.mult)
                    mrows[r // 4] = mrow4
                mrow4 = mrows[r // 4]
                moff = (r % 4) * N1
                vt = vpool.tile([2 * C, 612], BF16, tag="v")
                vts[r] = vt
                sup = ps.tile([2 * C, 1024], F32, tag="sup")
                for h2 in range(2):
                    for g in range(4):
                        w8 = h2 * 4 + g
                        off = (g // 2) * 512 + (g % 2) * 153
                        for ky in range(3):
                            slot = (r + ky) % XSL
                            st = xwb[:, slot * 512 + w8 * C:
                                     slot * 512 + (w8 + 1) * C]
                            mbase = mrow4[:, moff + (3 * ky) * WJ + w8 * CW:
                                          moff + (3 * ky) * WJ + w8 * CW + 1]
                            mv = _ap(mbase, [[WJ, 3], [1, CW]])
                            nc.tensor.matmul(
                                sup[h2 * C:(h2 + 1) * C,
                                    off + 51 * ky:off + 51 * (ky + 1)],
                                st, mv, start=True, stop=True,
                                tile_position=(0, h2 * C))
                # single strided evac [128, 612]: psum (b,s,run) -> vt
                sbase = sup[:, 0:1]
                src = _ap(sbase, [[512, 2], [153, 2], [1, 153]])
                if r % 2 == 0:
                    nc.vector.tensor_copy(vt[:], src)
                else:
                    nc.scalar.activation(vt[:], src, AF.Copy, bias=0.0)
                pend.append(r)
                if r >= 3:
                    stage2(pend.pop(0))
            while pend:
                stage2(pend.pop(0))
    nc.compile()
    return nc


def host_prep(input_feat, inter, weight, bias, w_om, b_om):
    import ml_dtypes
    bf = ml_dtypes.bfloat16
    maps = []
    womr = np.ascontiguousarray(
        w_om.transpose(1, 2, 3, 0).reshape(2 * C, 9 * 27)).astype(bf)
    w2r1 = np.ascontiguousarray(
        weight.reshape(O, C, K).transpose(1, 2, 0).reshape(C, K * O))
    w2r = np.concatenate([w2r1, w2r1], axis=0).astype(bf)   # h2-duplicated
    kk = (np.arange(K) % 3).astype(np.float32)
    jj = np.arange(CW, dtype=np.float32)
    # jkt tile layout: [72=(q,h,k), (r8, w8, j)] -> value dep on (k, j)
    jkt_t = np.zeros((72, 1088), np.float32)
    kwj = (3.0 + kk[:, None, None] + 0 * np.arange(NW)[None, :, None]
           + jj[None, None, :])                          # [9, 8, 17]
    for qh in range(8):
        for k in range(K):
            jkt_t[qh * 9 + k] = np.tile(kwj[k].reshape(1, NW * CW),
                                        (8, 1)).reshape(-1)
    dbias = np.tile(-(np.arange(5, dtype=np.float32) - 2), (72, 1))
    # x-window slot l ( = p % XW ) sits at t-coordinate l+1
    mxlb = -(np.arange(NP) % XW + 1).astype(np.float32)[:, None]
    for b in range(B):
        xpad = np.zeros((C, H + 6, 144), np.float32)
        xpad[:, 3:3 + H, 4:4 + W] = input_feat[b]
        featb = np.concatenate([input_feat[b], inter[b]], axis=0)
        for half in range(2):
            h0 = half * HH
            fs = np.zeros((2 * C, RG, 130), np.float32)
            r_lo, r_hi = max(0, h0 - 1), min(H, h0 + 65)
            fs[:, r_lo - (h0 - 1):r_hi - (h0 - 1), 1:129] = featb[:, r_lo:r_hi]
            xs = xpad[:, h0:h0 + 70, :]                    # [C, 70, 144]
            rows = (np.arange(RG)[:, None, None, None]
                    + np.arange(ND)[None, None, None, :])  # r + d
            cols = (17 * np.arange(NW)[None, :, None, None] + 1
                    + np.arange(XW)[None, None, :, None])
            xw = xs[:, rows, cols]                         # [C,66,NW,XW,ND]
            xw = xw.transpose(4, 3, 1, 2, 0).reshape(NP, RG * NW * C)
            maps.append({
                "feat": fs.reshape(2 * C, RG * 130).astype(bf),
                "xwin": xw.astype(bf),
                "womr": womr, "w2r": w2r,
                "jkt": jkt_t.astype(np.float16),
                "bias": np.asarray(bias, np.float32).reshape(O, 1),
                "bom": np.asarray(b_om, np.float32).reshape(27, 1),
                "dbias": dbias,
                "mxlb": mxlb,
            })
    return maps


def kernel(input_feat, inter, weight, bias, w_om, b_om):
    if "nc" not in _cache:
        _cache["nc"] = build_bass()
    nc = _cache["nc"]
    maps = host_prep(np.asarray(input_feat, np.float32),
                     np.asarray(inter, np.float32),
                     np.asarray(weight, np.float32),
                     np.asarray(bias, np.float32),
                     np.asarray(w_om, np.float32),
                     np.asarray(b_om, np.float32))
    res = run_bass_kernel_spmd(nc, maps, list(range(NCORES)))
    out = np.zeros((B, O, H, W), np.float32)
    for ci in range(NCORES):
        b, half = ci // 2, ci % 2
        out[b, :, half * HH:(half + 1) * HH] = \
            res.results[ci]["out"].astype(np.float32).reshape(O, HH, W)
    return out
